# revision 3
# baseline (speedup 1.0000x reference)
"""Deformable transformer encoder layer on 8 trn2 NeuronCores (Bass/Tile).

Sharding: core c = (batch c//4, head-pair c%4). Attention is head-sharded:
each core computes its 2 heads for all 21760 queries of its batch. Bilinear
sampling runs via dma_gather from a 4-parity-copy blocked value table in
DRAM (one 256B token = a full 2x2 bilinear patch x 32ch bf16, corners
[TL|BL|TR|BR]). Head-pair outputs are exchanged with an AllToAll within
each group of 4 cores; oproj + LN + FFN + LN run token-sharded (5440/core).
"""
import sys
for _p in ('/opt/trn_rl_repo',):
    if _p not in sys.path:
        sys.path.insert(0, _p)
import contextlib
import numpy as np
import ml_dtypes

import bass_rust
import concourse.bacc as bacc
import concourse.mybir as mybir
import concourse.tile as tile
from concourse.bass import AP
from concourse.bass_utils import run_bass_kernel_spmd
from concourse.masks import make_identity

bf = ml_dtypes.bfloat16
FP32, BF16, I16 = mybir.dt.float32, mybir.dt.bfloat16, mybir.dt.int16
ALU = mybir.AluOpType
ACTF = mybir.ActivationFunctionType
AXL = mybir.AxisListType

D_MODEL, D_FFN, N_LEVELS, N_POINTS = 256, 1024, 4, 4
SS = [(128, 128), (64, 64), (32, 32), (16, 16)]
LEVEL_START = [0, 16384, 20480, 21504]
LQ, BLK, NQT, H2 = 21760, 5440, 170, 2
import os as _os0
NQT_RUN = int(_os0.environ.get("NQT_RUN", "170"))
MAGIC = 12582912.0  # 1.5*2**23: fp32 round-to-nearest-int trick

TAB, _tok = [], 0
for (H, W) in SS:
    ny, nx = (H + 2) // 2 + 1, (W + 2) // 2 + 1
    TAB.append(dict(base=_tok, S=nx, T=ny * nx, ny=ny, nx=nx))
    _tok += 4 * ny * nx
TOK_PER_HEAD = _tok
N_TOK = 2 * TOK_PER_HEAD


def _split_excess_waits(nc, max_waits=1):
    for f in nc.m.functions:
        for blk in f.blocks:
            insts = blk.instructions
            i = 0
            while i < len(insts):
                inst = insts[i]
                si = getattr(inst, "sync_info", None)
                if si is not None and si.on_wait is not None and len(si.on_wait) > max_waits:
                    waits = list(si.on_wait)
                    si.on_wait.clear()
                    for w in waits[-max_waits:]:
                        si.on_wait.append(w)
                    for k, w in enumerate(waits[:-max_waits]):
                        nop = mybir.InstNoOp(
                            name=f"{inst.name}_xw{k}",
                            sync_info=mybir.SyncInfo(on_wait=[w], on_update=[]),
                            bass_nofuse=True, engine=inst.engine)
                        insts.insert(i, nop)
                        i += 1
                i += 1


class _TC(tile.TileContext):
    def _drain_and_barrier(self, tick_clock, wait_clock):
        nop_inst = self.nc.sync.nop(nofuse=True, hint="drain_waits")
        wait_clock.add_sem_waits(
            nop_inst.ins, bass_rust.ScopedClock({None: tick_clock.global_clock}))
        si = nop_inst.ins.sync_info
        waits = list(si.on_wait) if si is not None else []
        if si is not None:
            si.on_wait.clear()
        allocated = dict(self.sems.allocated())
        by_name = {h.name: h for h in allocated.values()}
        for w in waits:
            self.nc.sync.wait_ge(by_name[w.ant_name], w.wait_value)
        self.nc.sync.drain()
        self.nc.all_engine_barrier()
        popped = self.nc._tile_sem_poison_stack.pop()
        assert popped is self._sem_poison
        self.nc.clear_and_free_semaphores(list(allocated.values()))
        self.nc.all_engine_barrier()

    def __exit__(self, et, ev, tb):
        r = super().__exit__(et, ev, tb)
        if et is None:
            _split_excess_waits(self.nc)
        return r


def _host_consts():
    WH = np.zeros(64, np.float32)
    for h in range(H2):
        for l in range(N_LEVELS):
            for p in range(N_POINTS):
                b = ((h * N_LEVELS + l) * N_POINTS + p) * 2
                WH[b], WH[b + 1] = SS[l][1], SS[l][0]
    S_, T_, BASE = (np.zeros(32, np.float32) for _ in range(3))
    for h in range(H2):
        for l in range(N_LEVELS):
            for p in range(N_POINTS):
                k = (h * N_LEVELS + l) * N_POINTS + p
                S_[k], T_[k] = TAB[l]['S'], TAB[l]['T']
                BASE[k] = TAB[l]['base']
    RWH = np.zeros(8, np.float32)
    for l in range(N_LEVELS):
        RWH[2 * l], RWH[2 * l + 1] = SS[l][1], SS[l][0]
    rep = lambda v: np.tile(np.asarray(v, np.float32)[None, :], (128, 1))
    return dict(
        c_whm1=rep(WH - 0.5), c_whm2=rep(WH - 1.5), c_clmax=rep(WH),
        c_S=rep(S_), c_T=rep(T_), c_T2=rep(2 * T_), c_base=rep(BASE),
        c_rwh=rep(RWH), c_ones=rep(np.ones(64)))


_CACHE = {}


def _build():
    nc = bacc.Bacc("TRN2", debug=False, num_devices=8)
    ein = lambda n, s, d=FP32: nc.dram_tensor(n, s, d, kind="ExternalInput")
    src = ein("src", [LQ, D_MODEL])
    pos = ein("pos", [LQ, D_MODEL])
    refp = ein("refp", [LQ, 8])
    vproj_w2 = ein("vproj_w2", [D_MODEL, 64], BF16)
    offs_w2 = ein("offs_w2", [D_MODEL, 64], BF16)
    aw_w2 = ein("aw_w2", [D_MODEL, 32], BF16)
    oproj_w = ein("oproj_w", [512, D_MODEL], BF16)
    lin1_w = ein("lin1_w", [D_MODEL, D_FFN], BF16)
    lin2_w = ein("lin2_w", [D_FFN, D_MODEL], BF16)
    lin1_bT = ein("lin1_bT", [128, 8])
    rep_shapes = {"vproj_b2": 64, "offs_b2": 64, "aw_b2": 32, "oproj_b": 256,
                  "lin2_b": 256, "n1g": 256, "n1b": 256, "n2g": 256, "n2b": 256}
    reps = {n: ein(n, [128, s]) for n, s in rep_shapes.items()}
    hc = _host_consts()
    csts = {n: ein(n, [128, v.shape[1]]) for n, v in hc.items()}
    out_ext = nc.dram_tensor("out", [LQ, D_MODEL], FP32, kind="ExternalOutput")

    qbfT = nc.dram_tensor("qbfT", [D_MODEL, LQ], BF16)
    vrows = nc.dram_tensor("vrows", [LQ, 64], BF16)
    vtabs = [nc.dram_tensor(f"vtab{h}", [TOK_PER_HEAD, 128], BF16) for h in range(2)]
    wr16 = nc.dram_tensor("wr16", [16, 256], I16)
    ag_in = nc.dram_tensor("ag_in", [64, LQ], BF16)
    import os as _os
    ag_out = nc.dram_tensor("ag_out", [512, LQ], BF16,
                            addr_space="Local" if _os.environ.get("NO_COLL") == "1" else "Shared")

    with _TC(nc, num_cores=8) as tc, contextlib.ExitStack() as ctx:
        cons = ctx.enter_context(tc.tile_pool(name="cons", bufs=1))
        io = ctx.enter_context(tc.tile_pool(name="io", bufs=3))
        ps = ctx.enter_context(tc.tile_pool(name="ps", bufs=2, space="PSUM"))
        psb = ctx.enter_context(tc.tile_pool(name="psb", bufs=3, space="PSUM"))
        wp = ctx.enter_context(tc.tile_pool(name="wp", bufs=2))
        big = ctx.enter_context(tc.tile_pool(name="big", bufs=1))
        gp = ctx.enter_context(tc.tile_pool(name="gp", bufs=2))

        C = {}
        for n in list(csts) + list(rep_shapes):
            td = csts.get(n, reps.get(n))
            t = cons.tile([128, td.shape[1]], FP32, tag=f"c{n}")
            nc.sync.dma_start(out=t[:], in_=td[:])
            C[n] = t
        l1bT = cons.tile([128, 8], FP32, tag="l1bT")
        nc.sync.dma_start(out=l1bT[:], in_=lin1_bT[:])
        Wt = {}
        for n, td in [("vproj", vproj_w2), ("offs", offs_w2), ("aw", aw_w2),
                      ("oproj", oproj_w), ("lin1", lin1_w)]:
            t = cons.tile([128, td.shape[0] // 128, td.shape[1]], BF16, tag=f"w{n}")
            nc.sync.dma_start(out=t[:], in_=td[:, :].rearrange("(a b) c -> b a c", b=128))
            Wt[n] = t
        lin2_t = cons.tile([128, 8, 256], BF16, tag="wlin2")
        nc.sync.dma_start(out=lin2_t[:], in_=lin2_w[:, :].rearrange("(a b) c -> b a c", b=128))
        ident = cons.tile([128, 128], BF16, tag="ident")
        make_identity(nc, ident[:])

        def fap(t, d_off, dims):
            """AP into tile t: tile partition dim + custom free dims (elem offsets)."""
            base = t[:]
            return AP(base.tensor, base.offset + d_off, [base.ap[0]] + dims)

        # -------- pass 1: qbfT (transposed src+pos) and vrows ----------
        for i in range(NQT_RUN):
            sl = slice(i * 128, (i + 1) * 128)
            s_t = io.tile([128, 256], FP32, tag="p1s")
            nc.sync.dma_start(out=s_t[:], in_=src[sl, :])
            p_t = io.tile([128, 256], FP32, tag="p1p")
            nc.sync.dma_start(out=p_t[:], in_=pos[sl, :])
            q_t = io.tile([128, 256], BF16, tag="p1q")
            nc.vector.tensor_add(q_t[:], s_t[:], p_t[:])
            sb_t = io.tile([128, 256], BF16, tag="p1sb")
            nc.vector.tensor_copy(sb_t[:], s_t[:])
            vps = psb.tile([128, 64], FP32, tag="mm")
            for hf in range(2):
                pt = ps.tile([128, 128], BF16, tag="tp")
                nc.tensor.transpose(pt[:], q_t[:, hf * 128:(hf + 1) * 128], ident[:])
                ob = io.tile([128, 128], BF16, tag="p1o")
                nc.scalar.copy(ob[:], pt[:])
                nc.sync.dma_start(out=qbfT[hf * 128:(hf + 1) * 128, sl], in_=ob[:])
                pt2 = ps.tile([128, 128], BF16, tag="tp")
                nc.tensor.transpose(pt2[:], sb_t[:, hf * 128:(hf + 1) * 128], ident[:])
                ltb = io.tile([128, 128], BF16, tag="p2tb")
                nc.scalar.copy(ltb[:], pt2[:])
                nc.tensor.matmul(vps[:], lhsT=ltb[:], rhs=Wt["vproj"][:, hf, :],
                                 start=(hf == 0), stop=(hf == 1))
            vsb = io.tile([128, 64], BF16, tag="p2v")
            nc.vector.tensor_add(vsb[:], vps[:], C["vproj_b2"][:])
            nc.sync.dma_start(out=vrows[sl, :], in_=vsb[:])

        # -------- vtab: zero fill then blocked strided copies ----------
        zt = cons.tile([128, 1024], BF16, tag="zt")
        nc.vector.memset(zt[:], 0.0)
        _z = zt[:]
        for vt_ in vtabs:
            for r0 in range(0, TOK_PER_HEAD, 1024):
                na = min(1024, TOK_PER_HEAD - r0) // 128
                if na:
                    nc.sync.dma_start(
                        out=AP(vt_, r0 * 128, [[128, 128], [16384, na], [1, 128]]),
                        in_=AP(_z.tensor, _z.offset, [_z.ap[0], [128, na], [1, 128]]))
            rem = TOK_PER_HEAD % 1024
            r0 = TOK_PER_HEAD - rem
            while rem > 0:
                nn_ = min(128, rem)
                nc.sync.dma_start(
                    out=AP(vt_, r0 * 128, [[128, nn_], [1, 128]]),
                    in_=AP(_z.tensor, _z.offset, [[_z.ap[0][0], nn_], [1, 128]]))
                r0 += nn_
                rem -= nn_

        for l, (H, W) in enumerate(SS if _os0.environ.get("VT") != "0" else []):
            g = TAB[l]
            for py in range(2):
                for px in range(2):
                    cb = g['base'] + (py * 2 + px) * g['T']
                    for dy in range(2):
                        for dx in range(2):
                            oy, ox = py - 1 + dy, px - 1 + dx
                            Ylo, Xlo = (0 if oy >= 0 else 1), (0 if ox >= 0 else 1)
                            Yhi, Xhi = g['ny'] - 1, g['nx'] - 1
                            while 2 * Yhi + oy >= H:
                                Yhi -= 1
                            while 2 * Xhi + ox >= W:
                                Xhi -= 1
                            nY, nX = Yhi - Ylo + 1, Xhi - Xlo + 1
                            if nY <= 0 or nX <= 0:
                                continue
                            corner = dy + 2 * dx
                            for h in range(H2):
                                pix0 = LEVEL_START[l] + (2 * Ylo + oy) * W + (2 * Xlo + ox)
                                sap = AP(vrows, pix0 * 64 + h * 32,
                                         [[2 * W * 64, nY], [2 * 64, nX], [1, 32]])
                                tok0 = cb + Ylo * g['S'] + Xlo
                                dap = AP(vtabs[h], tok0 * 128 + corner * 32,
                                         [[g['S'] * 128, nY], [128, nX], [1, 32]])
                                nc.sync.dma_start(out=dap, in_=sap)

        # -------- attention pipeline ----------
        acc = big.tile([128, NQT, 64], FP32, tag="acc")
        W4all = big.tile([128, NQT, 2, 16, 4], BF16, tag="w4all")
        IDXF32 = big.tile([128, 2, NQT, 16], FP32, tag="idxf32")
        identf = cons.tile([128, 128], FP32, tag="identf")
        make_identity(nc, identf[:])

        for i in range(NQT_RUN):
            sl = slice(i * 128, (i + 1) * 128)
            pf1 = psb.tile([128, 64], FP32, tag="mm")
            pf2 = psb.tile([128, 32], FP32, tag="mm")
            for hf in range(2):
                qt = io.tile([128, 128], BF16, tag="p4q")
                nc.sync.dma_start(out=qt[:], in_=qbfT[hf * 128:(hf + 1) * 128, sl])
                nc.tensor.matmul(pf1[:], lhsT=qt[:], rhs=Wt["offs"][:, hf, :],
                                 start=(hf == 0), stop=(hf == 1))
                nc.tensor.matmul(pf2[:], lhsT=qt[:], rhs=Wt["aw"][:, hf, :],
                                 start=(hf == 0), stop=(hf == 1))
            taw = wp.tile([128, 32], FP32, tag="taw")
            nc.vector.tensor_add(taw[:], pf2[:], C["aw_b2"][:])
            tex = wp.tile([128, 32], FP32, tag="tex")
            nc.scalar.activation(tex[:], taw[:], ACTF.Exp)
            tsum = wp.tile([128, 2], FP32, tag="tsum")
            nc.vector.tensor_reduce(tsum[:], fap(tex, 0, [[16, 2], [1, 16]]),
                                    axis=AXL.X, op=ALU.add)
            trin = wp.tile([128, 2], FP32, tag="trin")
            nc.vector.reciprocal(trin[:], tsum[:])
            attn = wp.tile([128, 32], FP32, tag="attn")
            nc.vector.tensor_tensor(attn[:], tex[:], fap(trin, 0, [[1, 2], [0, 16]]),
                                    op=ALU.mult)
            toff = wp.tile([128, 64], FP32, tag="toff")
            nc.vector.tensor_add(toff[:], pf1[:], C["offs_b2"][:])
            rt = wp.tile([128, 8], FP32, tag="rt")
            nc.sync.dma_start(out=rt[:], in_=refp[sl, :])
            rwh = wp.tile([128, 8], FP32, tag="rwh")
            nc.vector.tensor_tensor(rwh[:], rt[:], C["c_rwh"][:], op=ALU.mult)
            nc.vector.tensor_scalar(out=rwh[:], in0=rwh[:], scalar1=-0.5,
                                    scalar2=None, op0=ALU.add)
            xy = wp.tile([128, 64], FP32, tag="xy")
            for hh in range(2):
                nc.vector.tensor_tensor(
                    fap(xy, hh * 32, [[8, 4], [2, 4], [1, 2]]),
                    fap(toff, hh * 32, [[8, 4], [2, 4], [1, 2]]),
                    fap(rwh, 0, [[2, 4], [0, 4], [1, 2]]),
                    op=ALU.add)
            t1 = wp.tile([128, 64], FP32, tag="t1")
            nc.vector.tensor_scalar(out=t1[:], in0=xy[:], scalar1=MAGIC,
                                    scalar2=None, op0=ALU.add)
            t2 = wp.tile([128, 64], FP32, tag="t2")
            nc.vector.tensor_scalar(out=t2[:], in0=t1[:], scalar1=-MAGIC,
                                    scalar2=None, op0=ALU.add)
            gt = wp.tile([128, 64], FP32, tag="gt")
            nc.vector.tensor_tensor(gt[:], t2[:], xy[:], op=ALU.is_gt)
            f0 = wp.tile([128, 64], FP32, tag="f0")
            nc.vector.tensor_tensor(f0[:], t2[:], gt[:], op=ALU.subtract)
            fr = wp.tile([128, 64], FP32, tag="fr")
            nc.vector.tensor_tensor(fr[:], xy[:], f0[:], op=ALU.subtract)
            v0 = wp.tile([128, 64], FP32, tag="v0")
            nc.vector.tensor_scalar(out=v0[:], in0=f0[:], scalar1=-0.5,
                                    scalar2=None, op0=ALU.is_gt)
            c2t = wp.tile([128, 64], FP32, tag="c2t")
            nc.vector.tensor_tensor(c2t[:], C["c_whm1"][:], f0[:], op=ALU.is_gt)
            nc.vector.tensor_tensor(v0[:], v0[:], c2t[:], op=ALU.mult)
            v1 = wp.tile([128, 64], FP32, tag="v1")
            nc.vector.tensor_scalar(out=v1[:], in0=f0[:], scalar1=-1.5,
                                    scalar2=None, op0=ALU.is_gt)
            nc.vector.tensor_tensor(c2t[:], C["c_whm2"][:], f0[:], op=ALU.is_gt)
            nc.vector.tensor_tensor(v1[:], v1[:], c2t[:], op=ALU.mult)
            A0 = wp.tile([128, 64], FP32, tag="A0")
            nc.vector.tensor_tensor(A0[:], C["c_ones"][:], fr[:], op=ALU.subtract)
            nc.vector.tensor_tensor(A0[:], A0[:], v0[:], op=ALU.mult)
            A1 = wp.tile([128, 64], FP32, tag="A1")
            nc.vector.tensor_tensor(A1[:], fr[:], v1[:], op=ALU.mult)
            xv = lambda t: fap(t, 0, [[2, 32]])
            yv = lambda t: fap(t, 1, [[2, 32]])
            t0 = wp.tile([128, 32], FP32, tag="t0")
            nc.vector.tensor_tensor(t0[:], attn[:], yv(A0), op=ALU.mult)
            tb1 = wp.tile([128, 32], FP32, tag="tb1")
            nc.vector.tensor_tensor(tb1[:], attn[:], yv(A1), op=ALU.mult)
            w4v = lambda c: fap(W4all, i * 128 + c, [[4, 32]])
            nc.vector.tensor_tensor(w4v(0), t0[:], xv(A0), op=ALU.mult)
            nc.vector.tensor_tensor(w4v(1), tb1[:], xv(A0), op=ALU.mult)
            nc.vector.tensor_tensor(w4v(2), t0[:], xv(A1), op=ALU.mult)
            nc.vector.tensor_tensor(w4v(3), tb1[:], xv(A1), op=ALU.mult)
            tp = wp.tile([128, 64], FP32, tag="tp")
            nc.vector.tensor_scalar(out=tp[:], in0=f0[:], scalar1=1.0, scalar2=0.0,
                                    op0=ALU.add, op1=ALU.max)
            nc.vector.tensor_tensor(tp[:], tp[:], C["c_clmax"][:], op=ALU.min)
            th = wp.tile([128, 64], FP32, tag="th")
            nc.vector.tensor_scalar(out=th[:], in0=tp[:], scalar1=0.5,
                                    scalar2=None, op0=ALU.mult)
            nc.vector.tensor_scalar(out=t1[:], in0=th[:], scalar1=MAGIC,
                                    scalar2=None, op0=ALU.add)
            nc.vector.tensor_scalar(out=t2[:], in0=t1[:], scalar1=-MAGIC,
                                    scalar2=None, op0=ALU.add)
            nc.vector.tensor_tensor(gt[:], t2[:], th[:], op=ALU.is_gt)
            F = wp.tile([128, 64], FP32, tag="F")
            nc.vector.tensor_tensor(F[:], t2[:], gt[:], op=ALU.subtract)
            par = wp.tile([128, 64], FP32, tag="par")
            nc.vector.scalar_tensor_tensor(par[:], F[:], -2.0, tp[:],
                                           op0=ALU.mult, op1=ALU.add)
            idxf = wp.tile([128, 32], FP32, tag="idxf")
            nc.vector.tensor_tensor(idxf[:], xv(par), C["c_T"][:], op=ALU.mult)
            i2 = wp.tile([128, 32], FP32, tag="i2")
            nc.vector.tensor_tensor(i2[:], yv(par), C["c_T2"][:], op=ALU.mult)
            nc.vector.tensor_tensor(idxf[:], idxf[:], i2[:], op=ALU.add)
            nc.vector.tensor_tensor(i2[:], yv(F), C["c_S"][:], op=ALU.mult)
            nc.vector.tensor_tensor(idxf[:], idxf[:], i2[:], op=ALU.add)
            nc.vector.tensor_tensor(idxf[:], idxf[:], xv(F), op=ALU.add)
            nc.vector.tensor_tensor(idxf[:], idxf[:], C["c_base"][:], op=ALU.add)
            nc.vector.tensor_scalar(
                out=fap(IDXF32, i * 16, [[NQT * 16, 2], [1, 16]]),
                in0=idxf[:], scalar1=0.0, scalar2=float(TOK_PER_HEAD - 1),
                op0=ALU.max, op1=ALU.min)

        # -------- gather + weighted reduce ----------
        CH = 2
        for c0 in range(0, NQT_RUN if _os0.environ.get("GA") != "0" else 0, CH):
            ntile = min(CH, NQT - c0)
            for h in range(H2):
                K = ntile * 16
                NI = 128 * K
                # fold idx [128p, K] -> wrapped [16, K*8]: PE transpose,
                # DVE free-shuffle (m,w)->(w,m) + int16 cast, then 2 DMAs.
                ptx = ps.tile([32, 128], FP32, tag="ptx")
                nc.tensor.matmul(ptx[:K, :],
                                 lhsT=fap(IDXF32, h * NQT * 16 + c0 * 16,
                                          [[1, K]]),
                                 rhs=identf[:], is_transpose=True)
                xt2 = gp.tile([32, 128], I16, tag="xt2")
                _x = xt2[:]
                _p = ptx[:]
                nc.vector.tensor_copy(
                    AP(_x.tensor, _x.offset, [[_x.ap[0][0], K], [8, 16], [1, 8]]),
                    AP(_p.tensor, _p.offset, [[_p.ap[0][0], K], [1, 16], [16, 8]]))
                nc.sync.dma_start(
                    out=AP(wr16, 0, [[8, K], [256, 16], [1, 8]]),
                    in_=AP(_x.tensor, _x.offset, [[_x.ap[0][0], K], [8, 16], [1, 8]]))
                idxw = gp.tile([128, K * 8], I16, tag="idxw")
                for g8 in range(8):
                    nc.sync.dma_start(
                        out=idxw[g8 * 16:(g8 + 1) * 16, :],
                        in_=AP(wr16, 0, [[256, 16], [1, K * 8]]))
                G = gp.tile([128, K, 128], BF16, tag="G")
                if _os0.environ.get("SKIPG") == "1":
                    nc.vector.memset(G[:], 0.0)
                elif True:
                    nc.gpsimd.dma_gather(
                        G[:], vtabs[h][:, :],
                        idxw[:], NI, NI, 128, single_packet=False)
                WG = gp.tile([128, K, 128], BF16, tag="WG")
                for qt in range(ntile):
                    nc.vector.tensor_tensor(
                        fap(WG, qt * 16 * 128, [[128, 16], [4, 32], [1, 4]]),
                        fap(G, qt * 16 * 128, [[128, 16], [1, 32], [32, 4]]),
                        fap(W4all, (c0 + qt) * 128 + h * 64, [[4, 16], [0, 32], [1, 4]]),
                        op=ALU.mult)
                    nc.vector.tensor_reduce(
                        fap(acc, (c0 + qt) * 64 + h * 32, [[1, 32]]),
                        fap(WG, qt * 16 * 128, [[4, 32], [128, 16], [1, 4]]),
                        axis=AXL.XY, op=ALU.add)

        # -------- transpose acc -> a2a_in ; AllToAll ----------
        for i in range(NQT_RUN):
            accb = io.tile([128, 64], BF16, tag="accb")
            nc.vector.tensor_copy(accb[:], acc[:, i, :])
            pt = ps.tile([128, 128], BF16, tag="tp")
            nc.tensor.transpose(pt[:64, :], accb[:], ident[:])
            ob = io.tile([64, 128], BF16, tag="a2o")
            nc.scalar.copy(ob[:], pt[:64, :])
            nc.sync.dma_start(out=ag_in[:, i * 128:(i + 1) * 128], in_=ob[:])
        import os as _os
        if _os.environ.get("NO_COLL") != "1":
            nc.gpsimd.collective_compute(
                "AllGather", ALU.bypass,
                replica_groups=[[0, 1, 2, 3, 4, 5, 6, 7]],
                ins=[ag_in[:]], outs=[ag_out[:]])
        else:
            for rr in range(4):
                nc.sync.dma_start(out=ag_out[rr * 64:(rr + 1) * 64, :], in_=ag_in[:])

        # -------- downstream ----------
        nt = [128] * NQT_RUN
        q0s = [128 * i for i in range(NQT_RUN)]

        def layernorm(xin, gg, bb, xout, n):
            mu = wp.tile([128, 1], FP32, tag="mu")
            nc.vector.tensor_reduce(mu[:n, :], xin[:n, :], axis=AXL.X, op=ALU.add)
            nc.vector.tensor_scalar(out=mu[:n, :], in0=mu[:n, :],
                                    scalar1=1.0 / 256, scalar2=None, op0=ALU.mult)
            xc = wp.tile([128, 256], FP32, tag="xc")
            nc.vector.tensor_scalar(out=xc[:n, :], in0=xin[:n, :],
                                    scalar1=mu[:n, :], scalar2=None, op0=ALU.subtract)
            sq = wp.tile([128, 256], FP32, tag="sq")
            nc.vector.tensor_tensor(sq[:n, :], xc[:n, :], xc[:n, :], op=ALU.mult)
            var = wp.tile([128, 1], FP32, tag="var")
            nc.vector.tensor_reduce(var[:n, :], sq[:n, :], axis=AXL.X, op=ALU.add)
            nc.vector.tensor_scalar(out=var[:n, :], in0=var[:n, :],
                                    scalar1=1.0 / 256, scalar2=1e-5,
                                    op0=ALU.mult, op1=ALU.add)
            sd = wp.tile([128, 1], FP32, tag="sd")
            nc.scalar.activation(sd[:n, :], var[:n, :], ACTF.Sqrt)
            rs = wp.tile([128, 1], FP32, tag="rs")
            nc.vector.reciprocal(rs[:n, :], sd[:n, :])
            nc.vector.tensor_scalar(out=xc[:n, :], in0=xc[:n, :],
                                    scalar1=rs[:n, :], scalar2=None, op0=ALU.mult)
            nc.vector.tensor_tensor(xc[:n, :], xc[:n, :], gg[:n, :], op=ALU.mult)
            nc.vector.tensor_add(xout[:n, :], xc[:n, :], bb[:n, :])

        for q0, n in zip(q0s, nt):
            po = psb.tile([128, 256], FP32, tag="mm")
            for hf in range(4):
                lt = io.tile([128, 128], BF16, tag="d0l")
                nc.sync.dma_start(out=lt[:, :n],
                                  in_=ag_out[hf * 128:(hf + 1) * 128, q0:q0 + n])
                nc.tensor.matmul(po[:n, :], lhsT=lt[:, :n],
                                 rhs=Wt["oproj"][:, hf, :],
                                 start=(hf == 0), stop=(hf == 3))
            x0 = wp.tile([128, 256], FP32, tag="x0")
            nc.vector.tensor_add(x0[:n, :], po[:n, :], C["oproj_b"][:n, :])
            sblk = io.tile([128, 256], FP32, tag="sblk")
            nc.sync.dma_start(out=sblk[:n, :], in_=src[q0:q0 + n, :])
            nc.vector.tensor_add(x0[:n, :], x0[:n, :], sblk[:n, :])
            x1 = wp.tile([128, 256], FP32, tag="x1")
            layernorm(x0, C["n1g"], C["n1b"], x1, n)
            x1b = wp.tile([128, 256], BF16, tag="x1b")
            nc.vector.tensor_copy(x1b[:n, :], x1[:n, :])
            x1T = wp.tile([128, 2, 128], BF16, tag="x1T")
            for hf in range(2):
                pt = ps.tile([128, 128], BF16, tag="tp")
                nc.tensor.transpose(pt[:, :n], x1b[:n, hf * 128:(hf + 1) * 128], ident[:])
                nc.scalar.copy(x1T[:, hf, :n], pt[:, :n])
            hT = wp.tile([128, 8, 128], BF16, tag="hT")
            for m in range(8):
                ph = ps.tile([128, 128], FP32, tag="tp")
                for hf in range(2):
                    nc.tensor.matmul(ph[:, :n],
                                     lhsT=Wt["lin1"][:, hf, m * 128:(m + 1) * 128],
                                     rhs=x1T[:, hf, :n],
                                     start=(hf == 0), stop=(hf == 1))
                nc.scalar.activation(hT[:, m, :n], ph[:, :n], ACTF.Relu,
                                     bias=l1bT[:, m:m + 1])
            x2 = wp.tile([128, 256], FP32, tag="x2")
            for hf in range(2):
                px2 = ps.tile([128, 128], FP32, tag="tp")
                for m in range(8):
                    nc.tensor.matmul(px2[:, :n],
                                     lhsT=lin2_t[:, m, hf * 128:(hf + 1) * 128],
                                     rhs=hT[:, m, :n],
                                     start=(m == 0), stop=(m == 7))
                x2Tb = wp.tile([128, 128], BF16, tag="x2Tb")
                nc.scalar.copy(x2Tb[:, :n], px2[:, :n])
                ptb = ps.tile([128, 128], BF16, tag="tp")
                nc.tensor.transpose(ptb[:n, :], x2Tb[:, :n], ident[:])
                nc.scalar.copy(x2[:n, hf * 128:(hf + 1) * 128], ptb[:n, :])
            nc.vector.tensor_add(x2[:n, :], x2[:n, :], C["lin2_b"][:n, :])
            nc.vector.tensor_add(x2[:n, :], x2[:n, :], x1[:n, :])
            xo = wp.tile([128, 256], FP32, tag="xo")
            layernorm(x2, C["n2g"], C["n2b"], xo, n)
            nc.sync.dma_start(out=out_ext[q0:q0 + n, :], in_=xo[:n, :])

    nc.compile()
    _split_excess_waits(nc)
    return nc


def kernel(**inputs):
    if 'nc' not in _CACHE:
        _CACHE['nc'] = _build()
    nc = _CACHE['nc']
    f32 = lambda x: np.asarray(x, np.float32)
    src = f32(inputs['src'])
    pos = f32(inputs['pos'])
    refp = f32(inputs['reference_points']).reshape(2, LQ, 8)
    rep = lambda v: np.tile(f32(v).reshape(-1)[None, :], (128, 1)).astype(np.float32)
    vw = f32(inputs['vproj_w']).reshape(256, 8, 32)
    vb = f32(inputs['vproj_b']).reshape(8, 32)
    ow = f32(inputs['offs_w']).reshape(256, 8, 32)
    ob = f32(inputs['offs_b']).reshape(8, 32)
    aw = f32(inputs['aw_w']).reshape(256, 8, 16)
    ab = f32(inputs['aw_b']).reshape(8, 16)
    hc = _host_consts()
    in_maps = []
    for c in range(8):
        b, j = c // 4, c % 4
        hs = slice(2 * j, 2 * j + 2)
        m = dict(
            src=src[b], pos=pos[b], refp=np.ascontiguousarray(refp[b]),
            vproj_w2=np.ascontiguousarray(vw[:, hs, :]).reshape(256, 64).astype(bf),
            offs_w2=np.ascontiguousarray(ow[:, hs, :]).reshape(256, 64).astype(bf),
            aw_w2=np.ascontiguousarray(aw[:, hs, :]).reshape(256, 32).astype(bf),
            oproj_w=np.concatenate([
                np.zeros((256 * b, 256), np.float32),
                f32(inputs['oproj_w']),
                np.zeros((256 * (1 - b), 256), np.float32)], 0).astype(bf),
            lin1_w=f32(inputs['lin1_w']).astype(bf),
            lin2_w=f32(inputs['lin2_w']).astype(bf),
            lin1_bT=np.ascontiguousarray(f32(inputs['lin1_b']).reshape(8, 128).T),
            vproj_b2=rep(vb[hs]), offs_b2=rep(ob[hs]), aw_b2=rep(ab[hs]),
            oproj_b=rep(inputs['oproj_b']), lin2_b=rep(inputs['lin2_b']),
            n1g=rep(inputs['norm1_g']), n1b=rep(inputs['norm1_b']),
            n2g=rep(inputs['norm2_g']), n2b=rep(inputs['norm2_b']),
        )
        m.update({k: np.ascontiguousarray(v, np.float32) for k, v in hc.items()})
        in_maps.append(m)
    _CACHE['in_maps'] = in_maps
    res = run_bass_kernel_spmd(nc, in_maps, list(range(8)))
    _CACHE['res'] = res
    out = np.stack([res.results[0]['out'], res.results[4]['out']])
    return out

